# revision 1
# baseline (speedup 1.0000x reference)
"""GNN message-passing layer (EGNN-style GCL) on 8 Trainium2 NeuronCores.

Strategy: shard edges across the 8 cores BY DESTINATION ROW (row =
edge_index[0], the segment-sum target); each core owns a 2500-node output
partition and its incoming edges, so no collectives are needed.

v2 pipeline (per core), all matmuls bf16, m1 kept feature-major:

  P0: comb_w[w] = [W1c_aug (17 rows); z1a_win (111 rows)]  per window w
        where z1a_win = h_win @ W_m1[0:128]    (built on PE, kept in SBUF)
      z1b_sb = h_all @ W_m1[128:256]           (bf16, SBUF-resident,
                                                node-wrapped for SBUF gather)
  P1 (edge loop, edges sorted by row into 111-node windows, tiles of 128):
      zcol_fm = dma_gather(z1b_sb, col, transpose=True)   # [128f, e] bf16
      m1_fm   = comb_w[w]^T @ [ea_aug; onehot_row]        # one matmul: W_m1
                + I^T @ zcol_fm                           # edge-attr + zrow
      m1e     = silu(m1_fm)                               # ACT, psum->sbuf
      m2      = m1e^T(lhsT) @ W_m2 -> edge-major psum; m2a = silu(m2)
      attr[e] = ttr: sum_f(m2a*Wa) + (b_a - 30*(1-mask))  # DVE, bf16 2x
      att     = 0.5*tanh(0.5*attr) + 0.5                  # ==sigmoid; tanh
                                                          # shares silu's ACT
                                                          # table (no reloads)
      oem     = (iota==rowloc) * att                      # scatter one-hot
      agg_fm[:, win] += m2a^T @ oem                       # PE matmul scatter
  P2 (node MLP, feature-major): as v1 but bf16 weights.

Pad edges carry rowloc=120 (>=111) so they scatter into discarded one-hot
columns; no mask multiply needed for padding.
"""
import sys

for _p in ('/opt/trn_rl_repo', '/root/.axon_site/_ro/trn_rl_repo'):
    if _p not in sys.path:
        sys.path.append(_p)

import numpy as np
import ml_dtypes

from concourse import bacc
import concourse.mybir as mybir
import concourse.tile as tile

F32 = mybir.dt.float32
BF16 = mybir.dt.bfloat16
I16 = mybir.dt.int16
F8 = mybir.dt.float8e4

# problem constants (hardcoded per spec)
N_NODES, N_EDGES = 20000, 640000
D, ED = 128, 16
NCORES = 8

DEFAULT_CFG = dict(
    n_nodes=N_NODES,
    npc=N_NODES // NCORES,      # 2500 nodes per core
    win=111,                    # nodes per scatter window (17+111=128 rows
                                # in the combined m1 lhsT)
    nw=23,                      # windows per core (23*111 = 2553 >= 2500)
    npad=2560,                  # P2 node padding (20 tiles of 128)
    ntbl=20096,                 # global node table rows (157*128)
    gch=24,                     # tiles per edge chunk
    grp=4,                      # tiles per m1/m2 psum group
)


# ----------------------------------------------------------------- host prep
def _host_prep(h, edge_index, edge_attr, edge_mask, cfg):
    """Bucket+sort edges by (core, window, col); build per-core arrays."""
    npc, win, nw = cfg['npc'], cfg['win'], cfg['nw']
    E = edge_index.shape[1]

    rowg = edge_index[0].astype(np.int64)
    colg = edge_index[1].astype(np.int64)
    core = rowg // npc
    rl = rowg - core * npc            # row local to core, [0, npc)
    w = rl // win                     # window within core
    slot = rl - w * win               # slot within window, [0, win)

    cnt = np.zeros((NCORES, nw), np.int64)
    np.add.at(cnt, (core, w), 1)
    tw = np.maximum(1, -(-cnt.max(axis=0) // 128))      # tiles per window
    off = np.concatenate([[0], np.cumsum(tw)])          # tile offset per window
    T = int(off[-1])
    TE = T * 128

    # within each (core, window) bucket order edges by column so the z1b
    # gather walks the table near-sequentially
    order = np.lexsort((colg, w, core))
    sc, sw = core[order], w[order]
    bucket = sc * nw + sw
    _, bstart, bcnt = np.unique(bucket, return_index=True, return_counts=True)
    pos = np.arange(E) - np.repeat(bstart, bcnt)
    eslot = off[sw] * 128 + pos       # slot within the core's edge stream

    tile_win = np.zeros(T, np.int64)
    for ww in range(nw):
        tile_win[off[ww]:off[ww + 1]] = ww

    ea_T = edge_attr.astype(np.float32).T               # [16, E]
    mask_v = edge_mask.astype(np.float32).reshape(-1)
    bf = ml_dtypes.bfloat16

    def wrap16(x):                    # element i -> [i%16, i//16], replicated x8
        return np.tile(np.ascontiguousarray(x.reshape(-1, 16).T), (8, 1))

    per_core = []
    for c in range(NCORES):
        m = sc == c
        sl = eslot[m]
        oi = order[m]
        zcol = np.zeros(TE, np.int16)
        zcol[sl] = colg[oi].astype(np.int16)
        # rhs_pack rows: 0:16 edge_attr^T, 16 bias ones, 17:128 row one-hot
        rhs = np.zeros((128, TE), np.float32)
        rhs[16, :] = 1.0
        rhs[:16, sl] = ea_T[:, oi]
        rhs[17 + slot[oi], sl] = 1.0
        rloc = np.full(TE, 120.0, np.float32)           # pads -> discard bin
        rloc[sl] = slot[oi].astype(np.float32)
        mk = np.zeros(TE, np.float32)                   # pad mask = 0
        mk[sl] = mask_v[oi]
        per_core.append(dict(
            zcol_idx=wrap16(zcol),
            rhs_pack=rhs.astype(ml_dtypes.float8_e4m3),
            rowloc_pm=np.ascontiguousarray(rloc.reshape(T, 128).T),
            mask_pm=np.ascontiguousarray(mk.reshape(T, 128).T),
        ))
    return per_core, T, tile_win


# ------------------------------------------------------------- device build
def _build(T, tile_win, cfg, act_fn):
    npc, win, nw = cfg['npc'], cfg['win'], cfg['nw']
    npad, ntbl = cfg['npad'], cfg['ntbl']
    gch, grp = cfg['gch'], cfg['grp']
    TE = T * 128
    nwp = npad // 128                 # P2 flag windows (128-aligned)
    nrank = ntbl // 128               # z1b table ranks

    nc = bacc.Bacc("TRN2", debug=False)

    # ---- inputs (packed to minimize HWDGE descriptor-gen serialization:
    # every dma_start pays ~650ns on the HWDGE regardless of size)
    # wpack_bf blocks: w1a, w2, wab, wn1h, wn2, ibf, iota
    wbf_d = nc.dram_tensor("wpack_bf", [128, 7 * D], BF16, kind="ExternalInput")
    # wpack_f32 blocks: wn1a(128), i32(128), bn1(1), bn2(1), flags(nwp)
    wf32_d = nc.dram_tensor("wpack_f32", [128, 2 * D + 2 + nwp], F32,
                            kind="ExternalInput")
    # f8pack blocks: comb_w0 (nw*D), w1b (D), hT_all (ntbl)
    f8_d = nc.dram_tensor("f8pack", [128, nw * D + D + ntbl], F8,
                          kind="ExternalInput")
    # hbf_pack blocks: hT_myw (nw*D), hT_my (npad)
    hbf_d = nc.dram_tensor("hbf_pack", [128, nw * D + npad], BF16,
                           kind="ExternalInput")
    # rm_pack blocks: rowloc (T), maskbias (T)
    rm_d = nc.dram_tensor("rm_pack", [128, 2 * T], F32, kind="ExternalInput")
    zcol_idx = nc.dram_tensor("zcol_idx", [128, TE // 16], I16, kind="ExternalInput")
    rhs_d = nc.dram_tensor("rhs_pack", [128, TE], F8, kind="ExternalInput")
    h_nm = nc.dram_tensor("h_nm", [npad, D], F32, kind="ExternalInput")

    out_d = nc.dram_tensor("out_nm", [npad, D], F32, kind="ExternalOutput")
    z1b_d = nc.dram_tensor("z1b_tbl", [ntbl, D], BF16)

    with tile.TileContext(nc) as tc:
        with (
            tc.tile_pool(name="consts", bufs=1) as cp,
            tc.tile_pool(name="streams", bufs=4) as sp,
            tc.tile_pool(name="small", bufs=14) as mp,
            tc.tile_pool(name="oemp", bufs=40) as op,
            tc.tile_pool(name="scrp", bufs=3) as scp,
            tc.tile_pool(name="node", bufs=2) as npool,
            tc.tile_pool(name="pm1", bufs=2, space="PSUM") as pm1,
            tc.tile_pool(name="pm2", bufs=2, space="PSUM") as pm2,
            tc.tile_pool(name="pmt", bufs=2, space="PSUM") as pmt,
            tc.tile_pool(name="pagg", bufs=2, space="PSUM") as pagg,
        ):
            # ---- load constants (few large packed DMAs)
            def cload(dram, shape, dt):
                t = cp.tile(shape, dt, tag=dram.name)
                nc.sync.dma_start(out=t[:], in_=dram[:])
                return t

            wbf = cload(wbf_d, [128, 7 * D], BF16)
            w1a, w2, wab, wn1h, wn2, ibf, iota = (
                wbf[:, i * D:(i + 1) * D] for i in range(7))
            wab4 = cp.tile([128, 4 * D], BF16, tag="wab4")
            for _i in range(4):
                nc.vector.tensor_copy(wab4[:, _i * D:(_i + 1) * D], wab)
            wf32 = cload(wf32_d, [128, 2 * D + 2 + nwp], F32)
            wn1a = wf32[:, 0:D]
            i32 = wf32[:, D:2 * D]
            bn1 = wf32[:, 2 * D:2 * D + 1]
            bn2 = wf32[:, 2 * D + 1:2 * D + 2]
            flagst = wf32[:, 2 * D + 2:2 * D + 2 + nwp]
            f8p = cp.tile([128, nw * D + D + ntbl], F8, tag="f8pack")
            _fw = nw * D + D + ntbl
            for _s in range(4):
                _lo = (_fw * _s // 4) // D * D
                _hi = (_fw * (_s + 1) // 4) // D * D if _s < 3 else _fw
                nc.sync.dma_start(out=f8p[:, _lo:_hi], in_=f8_d[:, _lo:_hi])
            combw = f8p[:, 0:nw * D]
            w1b = f8p[:, nw * D:nw * D + D]
            hTa = f8p[:, nw * D + D:]
            hbf = cload(hbf_d, [128, nw * D + npad], BF16)
            hTmyw = hbf[:, 0:nw * D]
            hTmy = hbf[:, nw * D:]
            rm = cload(rm_d, [128, 2 * T], F32)
            rowloc = rm[:, 0:T]
            mbias = rm[:, T:]
            zcix = cload(zcol_idx, [128, TE // 16], I16)

            agg_sb = cp.tile([D, npad], F32, tag="agg_sb")
            z1b_sb = cp.tile([128, nrank * D], BF16, tag="z1b_sb")
            # zero the agg tail beyond nw*win so P2 reads defined values
            if nw * win < npad:
                nc.vector.memset(agg_sb[:, nw * win:npad], 0.0)

            # ---------------- P0a: z1a windows into comb_w rows 17:128
            for w in range(nw):
                p = pagg.tile([128, 4 * D], F32, tag="agg")
                nc.tensor.matmul(p[:, 0:D], lhsT=hTmyw[:, w * D:(w + 1) * D],
                                 rhs=w1a[:], start=True, stop=True)
                # psum rows 0:16 are exactly zero (hT_myw has 17 zero lead
                # cols), so adding the full tile leaves the W1c rows intact
                nc.vector.tensor_tensor(out=combw[:, w * D:(w + 1) * D],
                                        in0=combw[:, w * D:(w + 1) * D],
                                        in1=p[:, 0:D],
                                        op=mybir.AluOpType.add)

            # ---------------- P0b: z1b table (node-wrapped bf16, SBUF)
            for r0 in range(0, nrank, 4):
                rn = min(4, nrank - r0)
                p = pm1.tile([128, 4 * D], F32, tag="m1")
                for j in range(rn):
                    nc.tensor.matmul(p[:, j * D:(j + 1) * D],
                                     lhsT=hTa[:, (r0 + j) * D:(r0 + j + 1) * D],
                                     rhs=w1b[:], start=(j == 0),
                                     stop=(j == rn - 1))
                # split the psum->sbuf converts across DVE and ACT
                # (GPSIMD cannot access PSUM on real hardware)
                dst = z1b_sb[:, r0 * D:(r0 + rn) * D]
                if (r0 // 4) % 2 == 0:
                    nc.vector.tensor_copy(dst, p[:, :rn * D])
                else:
                    nc.scalar.activation(dst, p[:, :rn * D],
                                         mybir.ActivationFunctionType.Copy)
                nc.sync.dma_start(
                    out=z1b_d[r0 * 128:(r0 + rn) * 128, :].rearrange(
                        "(r p) d -> p r d", p=128),
                    in_=z1b_sb[:, r0 * D:(r0 + rn) * D].rearrange(
                        "p (r d) -> p r d", d=D))

            # ---------------- P1: edge loop. The att chain (ttr -> tanh ->
            # attC -> oem -> scatter) for chunk i is emitted during chunk
            # i+1 (software pipeline): it otherwise head-of-line blocks the
            # ACT/PE/DVE queues at every chunk tail. oem generation is split
            # DVE/Pool to balance engine load.
            agg_p = None
            pend = []              # [(t0, ntc, m2a_list, attr)] two-deep queue

            def emit_att_oem(t0, ntc, m2a_list, attr):
                # att = sigmoid(attr) = 0.5*tanh(0.5*attr) + 0.5; tanh shares
                # silu's activation table so no table reloads occur
                attf = mp.tile([128, gch], F32, tag="attf")
                nc.vector.tensor_tensor(out=attf[:, :ntc],
                                        in0=attr[:, :ntc],
                                        in1=mbias[:, t0:t0 + ntc],
                                        op=mybir.AluOpType.add)
                th = mp.tile([128, gch], F32, tag="th")
                nc.scalar.activation(th[:, :ntc], attf[:, :ntc],
                                     mybir.ActivationFunctionType.Tanh,
                                     scale=0.5)
                attC = mp.tile([128, gch], F32, tag="attC")
                nc.vector.tensor_scalar(out=attC[:, :ntc], in0=th[:, :ntc],
                                        scalar1=0.5, scalar2=0.5,
                                        op0=mybir.AluOpType.mult,
                                        op1=mybir.AluOpType.add)
                oems = []
                for g0, gn, m2a in m2a_list:
                    for i in range(gn):
                        t = t0 + g0 + i
                        oem = op.tile([128, D], BF16, tag="oem")
                        eng = nc.vector
                        eng.tensor_scalar(
                            out=oem[:], in0=iota[:],
                            scalar1=rowloc[:, t:t + 1],
                            scalar2=attC[:, g0 + i:g0 + i + 1],
                            op0=mybir.AluOpType.is_equal,
                            op1=mybir.AluOpType.mult)
                        oems.append((t, m2a, slice(i * 128, (i + 1) * 128),
                                     oem))
                return oems

            done_wins = []         # (win, quad_tile, col0, emit_chunk)
            quad = {}              # current psum quad tile (4 windows/bank)

            def emit_scatter(oems, demote=True):
                # rank the scatter matmuls behind the MLP stream in the
                # scheduler's priority heap: they depend on the slow att
                # chain, and scheduled early they head-of-line block the PE.
                # Completed windows stay parked in their PSUM tile; the
                # psum->sbuf copy is deferred until just before the P2 group
                # that reads them (by then the scatters are long done, so
                # the copy never blocks the DVE queue).
                p0 = tc.cur_priority
                tc.cur_priority = p0 + (120 if demote else 0)
                for t, m2a, sl, oem in oems:
                    w_ = int(tile_win[t])
                    first = (t == 0) or (tile_win[t - 1] != w_)
                    last = (t == T - 1) or (tile_win[t + 1] != w_)
                    if first and w_ % 4 == 0:
                        aggq = pagg.tile([128, 4 * D], F32, tag="agg")
                        quad['t'] = aggq
                    c0 = (w_ % 4) * D
                    nc.tensor.matmul(quad['t'][:, c0:c0 + D],
                                     lhsT=m2a[:, sl], rhs=oem[:],
                                     start=first, stop=last)
                    if last and (w_ % 4 == 3 or w_ == nw - 1):
                        # whole quad complete: record one strided flush
                        done_wins.append((w_ - w_ % 4, w_ % 4 + 1,
                                          quad['t'], t // gch))
                tc.cur_priority = p0

            def flush_agg(upto_age=None):
                for w0, nq, tile_, ec in list(done_wins):
                    if upto_age is not None and ec > upto_age:
                        continue
                    nc.vector.tensor_copy(
                        agg_sb[:, w0 * win:(w0 + nq) * win].rearrange(
                            "p (q c) -> p q c", c=win),
                        tile_[:, :nq * 128].rearrange(
                            "p (q c) -> p q c", c=128)[:, :, 0:win])
                    done_wins.remove((w0, nq, tile_, ec))

            # ---------------- P2: node MLP (feature-major), emitted per
            # 512-node group as soon as the agg windows it reads are fully
            # scattered (hides the node MLP under the edge loop)
            def emit_p2(q0, demote=True):
                # demoted rank: P2 has slack; at normal rank its out-DMA
                # blocks the SP queue ahead of the rhs_c prefetches
                p0 = tc.cur_priority
                tc.cur_priority = p0 + (300 if demote else 0)
                flush_agg()
                qn = min(512, npad - q0)
                nb = qn // 128
                pu = pm1.tile([128, 512], F32, tag="m1")
                nc.tensor.matmul(pu[:, :qn], lhsT=wn1h,
                                 rhs=hTmy[:, q0:q0 + qn],
                                 start=True, stop=False)
                nc.tensor.matmul(pu[:, :qn], lhsT=wn1a,
                                 rhs=agg_sb[:, q0:q0 + qn],
                                 start=False, stop=True)
                u1 = npool.tile([128, 512], BF16, tag="u1")
                nc.scalar.activation(u1[:, :qn], pu[:, :qn], act_fn, bias=bn1)
                pup = pm2.tile([128, 512], F32, tag="m2")
                nc.tensor.matmul(pup[:, :qn], lhsT=wn2, rhs=u1[:, :qn],
                                 start=True, stop=True)
                updf = npool.tile([128, 512], F32, tag="updf")
                nc.scalar.activation(updf[:, :qn], pup[:, :qn],
                                     mybir.ActivationFunctionType.Identity,
                                     bias=bn2)
                ptr = pm2.tile([128, 512], F32, tag="m2")
                for i in range(nb):
                    nc.tensor.transpose(ptr[:, i * 128:(i + 1) * 128],
                                        updf[:, i * 128:(i + 1) * 128], i32)
                nh4 = npool.tile([128, 512], F32, tag="nh")
                nc.sync.dma_start(
                    out=nh4[:, :qn].rearrange("p (a d) -> p a d", d=128),
                    in_=h_nm[q0:q0 + qn, :].rearrange("(a p) d -> p a d",
                                                      p=128))
                so = npool.tile([128, 512], F32, tag="so")
                nc.vector.tensor_tensor(out=so[:, :qn], in0=ptr[:, :qn],
                                        in1=nh4[:, :qn],
                                        op=mybir.AluOpType.add)
                for i in range(nb):
                    blk = q0 // 128 + i
                    nc.vector.tensor_scalar(
                        out=so[:, i * 128:(i + 1) * 128],
                        in0=so[:, i * 128:(i + 1) * 128],
                        scalar1=flagst[:, blk:blk + 1],
                        scalar2=None, op0=mybir.AluOpType.mult)
                nc.sync.dma_start(
                    out=out_d[q0:q0 + qn, :].rearrange("(a p) d -> p a d",
                                                       p=128),
                    in_=so[:, :qn].rearrange("p (a d) -> p a d", d=128))
                tc.cur_priority = p0

            # P2 group q reads agg windows [q0//win, (q0+511)//win]; window w
            # is fully scattered two chunks after its last tile's chunk
            last_tile = {}
            for t in range(T):
                last_tile[int(tile_win[t])] = t
            p2_ready = {}
            p2_pending = []
            n_ci = (T + gch - 1) // gch
            for q0 in range(0, npad, 512):
                whi = min(nw - 1, ((q0 + 511) // win) | 3)
                p2_ready[q0] = last_tile[whi] // gch + 4
                if p2_ready[q0] >= n_ci:
                    p2_ready[q0] = -1          # after the drain
                p2_pending.append(q0)

            for t0 in range(0, T, gch):
                ntc = min(gch, T - t0)
                zcol_c = sp.tile([128, gch * 128], BF16, tag="zcol_c")
                for s0 in range(0, ntc, 8):
                    sn = min(8, ntc - s0)
                    nc.gpsimd.dma_gather(
                        zcol_c[:, s0 * 128:(s0 + sn) * 128].rearrange(
                            "p (o e) -> p o e", o=sn),
                        z1b_d[:],
                        zcix[:, (t0 + s0) * 8:(t0 + s0 + sn) * 8],
                        sn * 128, sn * 128, D)
                rhs_c = sp.tile([128, gch * 128], F8, tag="rhs_c")
                nc.sync.dma_start(out=rhs_c[:, :ntc * 128],
                                  in_=rhs_d[:, t0 * 128:(t0 + ntc) * 128])

                if len(pend) == 2:
                    oems = emit_att_oem(*pend.pop(0))
                else:
                    oems = None

                attr = mp.tile([128, gch], BF16, tag="attr")
                m2a_list = []

                def emit_m1(g0):
                    gn = min(grp, ntc - g0)
                    m1p = pm1.tile([128, grp * 128], F32, tag="m1")
                    for i in range(gn):
                        w = int(tile_win[t0 + g0 + i])
                        sl = slice(i * 128, (i + 1) * 128)
                        nc.tensor.matmul(
                            m1p[:, sl],
                            lhsT=rhs_c[:, (g0 + i) * 128:(g0 + i + 1) * 128],
                            rhs=combw[:, w * D:(w + 1) * D],
                            start=(i == 0), stop=False)
                    for i in range(gn):
                        sl = slice(i * 128, (i + 1) * 128)
                        nc.tensor.matmul(
                            m1p[:, sl],
                            lhsT=ibf[:],
                            rhs=zcol_c[:, (g0 + i) * 128:(g0 + i + 1) * 128],
                            start=False, stop=(i == gn - 1))
                    m1e = sp.tile([128, grp * 128], BF16, tag="m1e")
                    nc.scalar.activation(m1e[:, :gn * 128], m1p[:, :gn * 128],
                                         act_fn)
                    m1tp = pmt.tile([128, grp * 128], BF16, tag="mt")
                    for i in range(gn):
                        sl = slice(i * 128, (i + 1) * 128)
                        nc.tensor.matmul(m1tp[:, sl], lhsT=m1e[:, sl],
                                         rhs=ibf[:], is_transpose=True,
                                         start=(i == 0), stop=(i == gn - 1))
                    m1f = sp.tile([128, grp * 128], BF16, tag="m1f")
                    if (g0 // grp) % 2 == 0:
                        nc.vector.tensor_copy(m1f[:, :gn * 128],
                                              m1tp[:, :gn * 128])
                    else:
                        nc.scalar.activation(
                            m1f[:, :gn * 128], m1tp[:, :gn * 128],
                            mybir.ActivationFunctionType.Copy)
                    return m1f

                def emit_m2(g0, m1e):
                    gn = min(grp, ntc - g0)
                    gw = gn * 128
                    m2p = pm2.tile([128, grp * 128], F32, tag="m2")
                    for i in range(gn):
                        sl = slice(i * 128, (i + 1) * 128)
                        nc.tensor.matmul(m2p[:, sl], lhsT=m1e[:, sl], rhs=w2[:],
                                         start=(i == 0), stop=(i == gn - 1))
                    m2a = mp.tile([128, grp * 128], BF16, tag="m2a")
                    nc.scalar.activation(m2a[:, :gw], m2p[:, :gw], act_fn)
                    scrd = scp.tile([128, grp * 128], BF16, tag="scrd")
                    nc.vector.tensor_tensor(out=scrd[:, :gw], in0=m2a[:, :gw],
                                            in1=wab4[:, :gw],
                                            op=mybir.AluOpType.mult)
                    with nc.allow_low_precision(
                            reason="att logits tolerate bf16 rounding"):
                        nc.vector.tensor_reduce(
                            out=attr[:, g0:g0 + gn],
                            in_=scrd[:, :gw].rearrange("p (g d) -> p g d", d=D),
                            axis=mybir.AxisListType.X,
                            op=mybir.AluOpType.add)
                    m2a_list.append((g0, gn, m2a))

                # one-group software pipeline: m2(g) is emitted after m1(g+1)
                # so the PE never waits on ACT's silu round-trip; the prior
                # chunk's scatter batch slots in after two m1 groups, by which
                # time its oem stream (DVE) has run ahead of the PE
                prev = None
                for gi, g0 in enumerate(range(0, ntc, grp)):
                    m1e = emit_m1(g0)
                    if gi == 1 and oems is not None:
                        emit_scatter(oems)
                        oems = None
                    if prev is not None:
                        emit_m2(*prev)
                    prev = (g0, m1e)
                if oems is not None:        # short tail chunk
                    emit_scatter(oems)
                    oems = None
                if prev is not None:
                    emit_m2(*prev)

                pend.append((t0, ntc, m2a_list, attr))

                # interleave P2 groups whose agg windows completed scattering
                ci = t0 // gch
                flush_agg(upto_age=ci - 6)
                for q0 in list(p2_pending):
                    if p2_ready[q0] == ci:
                        emit_p2(q0)
                        p2_pending.remove(q0)
            # drain: demotion only hurts once no other work remains
            for p in pend:
                emit_scatter(emit_att_oem(*p), demote=False)
            for q0 in p2_pending:
                emit_p2(q0, demote=False)

    nc.compile()
    return nc


# --------------------------------------------------------------- entry point
_CACHE = {}


def kernel(h, edge_index, edge_attr, flags, edge_mask,
           W_m1, b_m1, W_m2, b_m2, W_a, b_a, W_n1, b_n1, W_n2, b_n2,
           cfg=None, act_fn=None, _sim=False, _sim_cores=None):
    """Full inputs in, full output out. Shards edges over 8 NeuronCores."""

    cfg = dict(DEFAULT_CFG, **(cfg or {}))
    if act_fn is None:
        act_fn = mybir.ActivationFunctionType.Silu
    npc, win, nw = cfg['npc'], cfg['win'], cfg['nw']
    npad, ntbl = cfg['npad'], cfg['ntbl']
    nwp = npad // 128
    n = h.shape[0]
    bf = ml_dtypes.bfloat16

    h = np.asarray(h, np.float32)
    edge_index = np.asarray(edge_index, np.int32)
    edge_attr = np.asarray(edge_attr, np.float32)
    flags = np.asarray(flags, np.float32)
    edge_mask = np.asarray(edge_mask, np.float32)

    per_core, T, tile_win = _host_prep(h, edge_index, edge_attr, edge_mask, cfg)

    key = (T, tuple(tile_win.tolist()), int(act_fn), n)
    if key not in _CACHE:
        _CACHE[key] = _build(T, tile_win, cfg, act_fn)
    nc = _CACHE[key]

    # b_m2 is all-zero in this problem's setup_inputs; the kernel does not
    # add it, so fail loudly if that ever changes.
    b_m2 = np.asarray(b_m2, np.float32)
    assert np.abs(b_m2).max() == 0.0, "b_m2 != 0 not supported by this kernel"
    b_a_f = float(np.asarray(b_a).reshape(-1)[0])

    hTg = np.zeros((D, ntbl), np.float32)
    hTg[:, :n] = h.T

    # comb_w0: W_m1 edge-attr block + bias row in rows 0:17, zeros below
    # (P0a fills rows 17:128 with the per-window z1a table)
    w1c_aug = np.vstack([np.asarray(W_m1)[2 * D:2 * D + ED],
                         np.asarray(b_m1)[None, :]]).astype(np.float32)
    combw0 = np.zeros((128, nw * D), np.float32)
    for w in range(nw):
        combw0[:17, w * D:(w + 1) * D] = w1c_aug

    f8 = ml_dtypes.float8_e4m3
    # wpack_bf blocks: w1a, w2, wab, wn1h, wn2, ibf, iota
    wpack_bf = np.concatenate([
        np.ascontiguousarray(np.asarray(W_m1)[0:D]).astype(np.float32),
        np.asarray(W_m2, np.float32),
        np.tile(np.asarray(W_a, np.float32).reshape(1, D), (D, 1)),
        np.ascontiguousarray(np.asarray(W_n1)[0:D]).astype(np.float32),
        np.asarray(W_n2, np.float32),
        np.eye(D, dtype=np.float32),
        np.tile(np.arange(D, dtype=np.float32), (D, 1)),
    ], axis=1).astype(bf)
    # f8pack blocks: comb_w0, w1b, hT_all
    f8pack = np.concatenate([
        combw0,
        np.ascontiguousarray(np.asarray(W_m1)[D:2 * D]).astype(np.float32),
        hTg,
    ], axis=1).astype(f8)

    in_maps = []
    for c in range(NCORES):
        base = c * npc
        hmy = h[base:base + npc]                      # [2500, 128]
        hT_my = np.zeros((D, npad), np.float32)
        hT_my[:, :npc] = hmy.T
        h_nm = np.zeros((npad, D), np.float32)
        h_nm[:npc] = hmy
        # per-window h with 17 leading zero columns (z1a lands on rows 17:128)
        hT_myw = np.zeros((D, nw * D), np.float32)
        for w in range(nw):
            lo = w * win
            hi = min(lo + win, npc)
            hT_myw[:, w * D + 17:w * D + 17 + hi - lo] = hmy.T[:, lo:hi]
        fl = np.zeros(npad, np.float32)
        fl[:npc] = flags.reshape(-1)[base:base + npc]
        # wpack_f32 blocks: wn1a, i32, bn1, bn2, flags
        wpack_f32 = np.concatenate([
            np.ascontiguousarray(np.asarray(W_n1)[D:2 * D]).astype(np.float32),
            np.eye(D, dtype=np.float32),
            np.asarray(b_n1, np.float32).reshape(D, 1),
            np.asarray(b_n2, np.float32).reshape(D, 1),
            np.ascontiguousarray(fl.reshape(nwp, 128).T),
        ], axis=1)
        pc = per_core[c]
        maskbias = b_a_f - 30.0 * (1.0 - pc['mask_pm'])
        in_maps.append(dict(
            wpack_bf=wpack_bf,
            wpack_f32=wpack_f32,
            f8pack=f8pack,
            hbf_pack=np.concatenate([hT_myw, hT_my], axis=1).astype(bf),
            rm_pack=np.concatenate(
                [pc['rowloc_pm'], maskbias], axis=1).astype(np.float32),
            h_nm=h_nm,
            zcol_idx=pc['zcol_idx'], rhs_pack=pc['rhs_pack'],
        ))

    if _sim:
        from concourse.bass_interp import CoreSim
        core_outs = [None] * NCORES
        for c in (_sim_cores if _sim_cores is not None else range(NCORES)):
            sim = CoreSim(nc)
            for k, v in in_maps[c].items():
                sim.tensor(k)[:] = v
            sim.simulate()
            core_outs[c] = np.array(sim.tensor("out_nm"))
    else:
        from concourse.bass_utils import run_bass_kernel_spmd
        res = run_bass_kernel_spmd(nc, in_maps, core_ids=list(range(NCORES)))
        core_outs = [res.results[c]["out_nm"] for c in range(NCORES)]

    out = np.zeros((n, D), np.float32)
    for c in range(NCORES):
        base = c * npc
        lim = min(npc, n - base)
        if core_outs[c] is not None:
            out[base:base + lim] = core_outs[c][:lim]
    return out

